# revision 45
# baseline (speedup 1.0000x reference)
"""Trainium2 Bass kernel for nn_CKConv (SIREN kernel-net + causal conv1d).

Decomposition (8 cores, SPMD — identical program, per-core data):
  z[n,o] = sum_{ci, l<=n} W[o,ci,4095-l] * x[n-l,ci],  W[o,ci,m]=weights[m,32o+ci]
Tap l = 128u + 16c + 4g + dl  (c = core, u in [0,32), g,dl in [0,4)).
Each core computes SIREN weights for its 512 taps (SIREN tile cols mm per
block Jb: tap (us,g,dl) at mm=16us+4g+dl, u=8Jb+us), writes layer-3 output
to DRAM, gathers it back as conv lhsT tiles [(32dl+ci), (us,g,o)], and runs
144 accumulating [K=128,M=128,N<=512] matmuls against a 4-shift x image XS.
psum row (g,o) of output tile T holds the contribution to
z[512T+dn+4g+16c, o]; the host sums the shifted partials.

Perf structure: PE warm-up matmuls at t=0 (p-state ramp), sin via the
Activation engine's native Sin after a 4-piece Cody-Waite mod-pi reduction
(parity sign on GPSIMD, off the DVE critical path), layer-3 blocks
interleaved into the conv wave emission so PE never idles on the
W->DRAM->lhsT gather latency, and triangular trim of diagonal conv blocks.

Numerics: layer-1 sin args (up to ~3.4e5 rad) use the reference's exact fp32
rounding sequence; r = a - round(a/pi)*pi via 3 exact 7-bit pieces + 1 fp32
tail (|r| <= pi/2 + 0.021), sin(a) = (-1)^k sin(r).
"""
import os
import sys
import numpy as np

if "/opt/trn_rl_repo" not in sys.path:
    sys.path.insert(0, "/opt/trn_rl_repo")

f32 = np.float32
OMEGA = 32.5
N, CIN, COUT, H = 4096, 32, 32, 32
NCORES = 8
PAD = 512
XSW = PAD + N + 4

MAGIC = 12582912.0  # 1.5 * 2^23
INVPI = float(np.float32(1.0 / np.pi))


def _pieces(val, nexact, bits):
    ps, resid = [], np.float64(val)
    for _ in range(nexact):
        x = np.float32(resid)
        m, e = np.frexp(x)
        q = np.float32(np.ldexp(np.round(np.ldexp(m, bits)), int(e) - bits))
        ps.append(float(q))
        resid = resid - np.float64(q)
    ps.append(float(np.float32(resid)))
    return ps


P1 = _pieces(np.pi, 3, 7)          # |k|<=2^17: 3 exact 7-bit pieces + tail
# |k|<=2^6: single rounded fl32(2pi) piece — residual k*9e-9 rad is
# far inside the sin2 error budget
P2TAU = [float(np.float32(2 * np.pi)), float(np.float32(2*np.pi - np.float64(np.float32(2*np.pi))))]
INV2PI = float(np.float32(1.0 / (2 * np.pi)))

NWARM = 24
NFILL1 = 25
NFILL2 = 45
_CACHE = {}


def _build_program():
    import concourse.bacc as bacc
    import concourse.mybir as mybir
    import concourse.tile as tile

    dt = mybir.dt.float32
    f16 = mybir.dt.float16
    AF = mybir.ActivationFunctionType
    OP = mybir.AluOpType

    nc = bacc.Bacc("TRN2", target_bir_lowering=False, debug=False,
                   num_devices=NCORES)

    d_cst = nc.dram_tensor("consts", [128, 164], dt, kind="ExternalInput")
    d_w3 = nc.dram_tensor("w3rep", [128, 1024], f16, kind="ExternalInput")
    d_b3 = nc.dram_tensor("b3rep", [128, 1024], f16, kind="ExternalInput")
    d_xT = nc.dram_tensor("xT", [32, 4096], f16, kind="ExternalInput")
    d_out = nc.dram_tensor("out", [8, 128, 512], f16, kind="ExternalOutput")

    with tile.TileContext(nc) as tc:
        with (
            tc.tile_pool(name="const", bufs=1) as cp,
            tc.tile_pool(name="sin", bufs=1) as sp,
            tc.tile_pool(name="wsb", bufs=2) as wp,
            tc.tile_pool(name="lhs", bufs=1) as lp,
            tc.tile_pool(name="osb", bufs=4) as op_,
            tc.tile_pool(name="ps3", bufs=4, space="PSUM") as p3p,
            tc.tile_pool(name="psc", bufs=1, space="PSUM") as ppc,
            tc.tile_pool(name="wdram", bufs=1, space="DRAM") as dp,
        ):
            # ---------- t=0: warm-up, act-table preload, memsets, input DMAs
            wz = cp.tile([128, 256], f16, tag="wz")
            nc.gpsimd.memset(wz[:], 0.0)
            zd = sp.tile([128, 1], dt, tag="zd")
            nc.gpsimd.memset(zd[:], 0.0)
            sd = sp.tile([128, 1], dt, tag="sd")
            nc.scalar.activation(sd[:], zd[:], AF.Sin)  # trig table preload
            # warm-up matmuls: keep the PE engine busy (and its p-state
            # ramped) from t~1us until layer-2 inputs arrive
            wps = ppc.tile([128, 512], dt, tag="c0", name="wps")
            for _ in range(NWARM):
                nc.tensor.matmul(wps[:, 0:256], wz[:, 0:128], wz[:, 0:256],
                                 start=True, stop=True)

            cst = cp.tile([128, 164], dt, tag="cst")
            nc.sync.dma_start(cst[:], d_cst[:])
            t1 = cst[:, 0:128]
            w1r = cst[:, 128:129]
            b1r = cst[:, 129:130]
            sc2 = cst[:, 130:131]
            ob2 = cst[:, 131:132]
            v2Tr = cst[:, 132:164]
            w3r = cp.tile([128, 1024], f16, tag="w3r")
            nc.sync.dma_start(w3r[:], d_w3[:])
            b3r = cp.tile([128, 1024], f16, tag="b3r")
            nc.sync.dma_start(b3r[:], d_b3[:])

            # XS: [128=(32dl+ci), XSW];  XS[(dl,ci), PAD+dl+j] = x[j,ci]
            xs = cp.tile([128, XSW], f16, tag="xs")
            nc.gpsimd.memset(xs[:, 0:516], 0.0)
            xs_r = xs[:].rearrange("(dl ci) w -> dl ci w", dl=4)
            for dl in range(4):
                nc.sync.dma_start(
                    xs_r[dl, :, PAD + dl:PAD + dl + 4096], d_xT[:])

            # ---------- sin: DVE Cody-Waite reduce, Act-engine native Sin.
            # mode "parity": reduce mod pi, sign via sin(r*s) = s*sin(r)
            #   with s = (-1)^k, parity from k mod 2 (d^2 is robust to
            #   fmod-vs-floormod negative conventions).
            # mode "clamp": reduce mod 2pi (k slop ~1e-5 rad), clamp to
            #   +-fl32(pi)-ulp so the Sin table range holds.
            PI_IN = float(np.nextafter(np.float32(np.pi), np.float32(0)))

            def dev_sin(out, arg, pieces, inv, nm, parity):
                v = sp.tile([128, 128], dt, tag=f"{nm}v")
                k = sp.tile([128, 128], dt, tag=f"{nm}k")
                r = sp.tile([128, 128], dt, tag=f"{nm}r")
                t_ = sp.tile([128, 128], dt, tag=f"{nm}t")
                nc.vector.tensor_scalar(v[:], arg, inv, MAGIC,
                                        op0=OP.mult, op1=OP.add)
                nc.vector.tensor_scalar_sub(k[:], v[:], MAGIC)
                # piece chain and parity chain are independent given k;
                # interleave them so the in-order DVE pipeline overlaps the
                # dependent-op latencies of both chains.
                piece_ops = []
                sgn = 1.0
                for i, p in enumerate(pieces):
                    dst = r if i % 2 == 0 else t_
                    src = arg if i == 0 else (r if i % 2 == 1 else t_)[:]
                    piece_ops.append((dst, sgn * float(p), src))
                    sgn = -sgn
                cur = (r if (len(pieces) - 1) % 2 == 0 else t_)[:]
                par_ops = []
                if parity:
                    d = sp.tile([128, 128], dt, tag=f"{nm}d")
                    s = sp.tile([128, 128], dt, tag=f"{nm}s")
                    # d = k-2*round(k/2) in {-1,0,1}; s = 1-2*d^2 = (-1)^k
                    par_ops = [
                        lambda: nc.vector.tensor_scalar(
                            d[:], k[:], 0.5, MAGIC, op0=OP.mult, op1=OP.add),
                        lambda: nc.vector.tensor_scalar_sub(d[:], d[:], MAGIC),
                        lambda: nc.vector.scalar_tensor_tensor(
                            d[:], d[:], -2.0, k[:], op0=OP.mult, op1=OP.add),
                        lambda: nc.vector.tensor_mul(d[:], d[:], d[:]),
                        lambda: nc.vector.tensor_scalar(
                            s[:], d[:], -2.0, 1.0, op0=OP.mult, op1=OP.add),
                    ]
                ni = max(len(piece_ops), len(par_ops))
                for i in range(ni):
                    if i < len(par_ops):
                        par_ops[i]()
                    if i < len(piece_ops):
                        dst, p, src = piece_ops[i]
                        nc.vector.scalar_tensor_tensor(
                            dst[:], k[:], p, src,
                            op0=OP.mult, op1=OP.subtract)
                if parity:
                    nc.vector.tensor_mul(cur, cur, s[:])
                else:
                    nc.vector.tensor_scalar(cur, cur, PI_IN, -PI_IN,
                                            op0=OP.min, op1=OP.max)
                nc.scalar.activation(out, cur, AF.Sin)

            # ---------- layer 1:  a1 = fl(fl(fl(t*w1)+b1)*OMEGA)
            a1 = sp.tile([128, 128], dt, tag="a1")
            nc.vector.tensor_scalar(a1[:], t1, w1r, b1r,
                                    op0=OP.mult, op1=OP.add)
            nc.vector.tensor_scalar_mul(a1[:], a1[:], OMEGA)
            h1q = cp.tile([128, 128], dt, tag="h1q")
            dev_sin(h1q[:], a1[:], P1, INVPI, "s1", parity=True)

            # ---------- layer 2 (sca2 = OMEGA*g2/||v2|| from host)
            ps2 = p3p.tile([128, 512], dt, tag="ps3", name="ps2")
            for q in range(4):
                nc.tensor.matmul(ps2[32 * q:32 * q + 32, 0:128],
                                 v2Tr[32 * q:32 * q + 32],
                                 h1q[32 * q:32 * q + 32, :],
                                 start=True, stop=True,
                                 tile_position=(32 * q, 32 * q))
            a2 = sp.tile([128, 128], dt, tag="a2")
            nc.vector.tensor_scalar(a2[:], ps2[:, 0:128], sc2, ob2,
                                    op0=OP.mult, op1=OP.add)
            h2q = cp.tile([128, 128], f16, tag="h2q")
            dev_sin(h2q[:], a2[:], P2TAU, INV2PI, "s2", parity=False)

            # ---------- layer 3 + DRAM-roundtrip gather + conv
            lb = [lp.tile([128, 1024], f16, tag=f"lb{j}", name=f"lb{j}")
                  for j in range(4)]
            wd = [dp.tile([128, 1024], f16, tag=f"wd{j}", name=f"wd{j}")
                  for j in range(4)]

            def l3(Jb):
                pa = p3p.tile([128, 512], dt, tag="ps3")
                pb = p3p.tile([128, 512], dt, tag="ps3")
                for fb, pp in ((0, pa), (1, pb)):
                    nc.tensor.matmul(pp[:],
                                     h2q[32 * Jb:32 * Jb + 32, :],
                                     w3r[32 * Jb:32 * Jb + 32,
                                         512 * fb:512 * fb + 512],
                                     start=True, stop=True,
                                     tile_position=(32 * Jb, 0))
                return pa, pb

            def evac(Jb, pa, pb, engw, engg):
                # wsb = ps3 + b3 (fp16); contiguous wd write per column-half
                # (each starts as soon as its badd lands), then a strided
                # gather back, split by dl-halves across two DMA queues:
                # lb[(32dl+ci), 128us+32g+o] = wd[(16us+4g+dl), 32ci+o]
                # Blocked dma_starts sit on engw/engg's sequencer, so those
                # queues must hold nothing later that is needed earlier.
                wsb = wp.tile([128, 1024], f16, tag=f"wsb{Jb % 2}")
                gsrc = wd[Jb][:].rearrange(
                    "(us g dl) (ci o) -> dl ci us g o", g=4, dl=4, o=32)
                nc.vector.tensor_add(wsb[:, 0:512], pa[:], b3r[:, 0:512])
                if engg is None:
                    engw.dma_start(wd[Jb][:, 0:512], wsb[:, 0:512])
                    nc.vector.tensor_add(wsb[:, 512:1024], pb[:],
                                         b3r[:, 512:1024])
                    engw.dma_start(wd[Jb][:, 512:1024], wsb[:, 512:1024])
                    engw.dma_start(lb[Jb][:], gsrc)
                else:
                    # latency-critical Jb0: disjoint wd writes — a tiny
                    # rows[0:16] (us=0) full-width write plus rows[16:128]
                    # column-halves — so the mini-gather of the first
                    # u-slice waits only on the 32KB write and the conv's
                    # first matmuls start two DMA-stages earlier; the bulk
                    # gather halves exclude us=0 so the u=0 Ldweights only
                    # waits on the mini-gather
                    enga, engb = engg
                    engw.dma_start(wd[Jb][16:128, 0:512], wsb[16:128, 0:512])
                    nc.vector.tensor_add(wsb[:, 512:1024], pb[:],
                                         b3r[:, 512:1024])
                    engw.dma_start(wd[Jb][0:16, :], wsb[0:16, :])
                    engw.dma_start(wd[Jb][16:128, 512:1024],
                                   wsb[16:128, 512:1024])
                    engb.dma_start(lb[Jb][:, 0:128], gsrc[:, :, 0:1])
                    enga.dma_start(lb[Jb][0:64, 128:1024], gsrc[0:2, :, 1:8])
                    engb.dma_start(lb[Jb][64:128, 128:1024],
                                   gsrc[2:4, :, 1:8])

            pscs = {}
            # out-DMA queues: spread so the tail T5/T6/T7 outs don't
            # serialize on one sequencer
            # out DMAs never share a queue with the osb psum-copies (Act):
            # a blocked out-DMA would stall later copies and the B-wave's
            # psum-bank reuse behind them
            # out DMAs never share a queue with the osb psum-copies (Act):
            # a blocked out-DMA would stall later copies and the B-wave's
            # psum-bank reuse behind them.  Pool's SWDGE is slow (~1us
            # descriptor gen) so the tail tiles go on SP.
            oqueue = {0: nc.sync, 1: nc.sync, 2: nc.sync, 3: nc.sync,
                      4: nc.gpsimd, 5: nc.gpsimd, 6: nc.sync, 7: nc.scalar}

            def conv_wave(Ts, us_range):
                for u in us_range:
                    Jb, us = u // 8, u % 8
                    lt = lb[Jb][:, 128 * us:128 * us + 128]
                    for T in Ts:
                        nu_T = 4 * (T + 1)
                        if u >= nu_T:
                            continue
                        if T not in pscs:
                            pscs[T] = ppc.tile([128, 512], dt, tag=f"c{T % 4}",
                                               name=f"psc{T}")
                        dn0 = max(0, 128 * (u - 4 * T))
                        off = PAD + 512 * T - 128 * u
                        nc.tensor.matmul(pscs[T][:, dn0:512], lt,
                                         xs[:, off + dn0:off + 512],
                                         start=(u == 0), stop=(u == nu_T - 1),
                                         skip_group_check=True)
                        if u == nu_T - 1:
                            # psum evac on the (otherwise idle) Act engine:
                            # a DVE copy would head-of-line block later badds
                            osb = op_.tile([128, 512], f16, tag=f"o{T % 4}")
                            nc.scalar.activation(osb[:], pscs[T][:],
                                                 AF.Copy)
                            oqueue[T].dma_start(d_out[T], osb[:])

            # filler matmuls keep the PE engine streaming through the psum
            # buffer-rotation waits and the lb0 gather latency, so conv
            # matmuls are dispatched (and costed) against a warm engine
            def filler(n, name):
                fps = ppc.tile([128, 512], dt, tag="c0", name=name)
                for _ in range(n):
                    nc.tensor.matmul(fps[:, 0:256], wz[:, 0:128],
                                     wz[:, 0:256], start=True, stop=True)

            pa0, pb0 = l3(0)
            pa1, pb1 = l3(1)
            evac(0, pa0, pb0, nc.sync, (nc.scalar, nc.sync))
            evac(1, pa1, pb1, nc.sync, None)
            filler(NFILL1, "fps1")
            pa2, pb2 = l3(2)
            pa3, pb3 = l3(3)
            evac(2, pa2, pb2, nc.sync, None)
            evac(3, pa3, pb3, nc.sync, None)
            filler(NFILL2, "fps2")
            conv_wave((0, 1, 2, 3), range(0, 16))
            conv_wave((4, 5, 6, 7), range(0, 32))

    nc.finalize()
    return nc


def _host_prep(inputs):
    """Per-core input maps.  Only consts (t1 packing) differs across cores."""
    import jax
    import jax.numpy as jnp
    cpu = jax.devices("cpu")[0]
    with jax.default_device(cpu):
        t_new = np.asarray(jnp.linspace(-1.0, 2.0 * (N / 1.0) - 1.0, N))
    t_new = t_new.astype(f32)

    x = np.asarray(inputs["x"], dtype=f32)
    v1 = np.asarray(inputs["v1"], dtype=f32)
    g1 = np.asarray(inputs["g1"], dtype=f32)
    b1 = np.asarray(inputs["b1"], dtype=f32)
    v2 = np.asarray(inputs["v2"], dtype=f32)
    g2 = np.asarray(inputs["g2"], dtype=f32)
    b2 = np.asarray(inputs["b2"], dtype=f32)
    w3 = np.asarray(inputs["w3"], dtype=f32)
    b3 = np.asarray(inputs["b3"], dtype=f32)

    w1 = (g1[:, None] * v1 / np.linalg.norm(v1, axis=1, keepdims=True)
          ).astype(f32)[:, 0]
    sca2 = (OMEGA * g2 / np.linalg.norm(v2, axis=1)).astype(f32)
    ob2 = (OMEGA * b2).astype(f32)

    common = {
        "w3rep": np.ascontiguousarray(np.tile(
            np.transpose(w3.reshape(COUT, CIN, H), (2, 1, 0)
                         ).reshape(H, CIN * COUT), (4, 1))).astype(np.float16),
        "b3rep": np.ascontiguousarray(np.tile(
            b3.reshape(COUT, CIN).T.reshape(-1), (128, 1))).astype(np.float16),
        "xT": np.ascontiguousarray(x.T).astype(np.float16),
    }

    mm = np.arange(128)
    g_of = (mm % 16) // 4
    dl_of = mm % 4
    in_maps = []
    for c in range(NCORES):
        t1 = np.empty((128, 128), dtype=f32)
        for Jb in range(4):
            u = 8 * Jb + mm // 16
            m_of = 4095 - 128 * u - 16 * c - 4 * g_of - dl_of
            t1[32 * Jb:32 * Jb + 32, :] = t_new[m_of][None, :]
        cstm = np.concatenate([
            t1,
            np.tile(w1, 4)[:, None], np.tile(b1, 4)[:, None],
            np.tile(sca2, 4)[:, None], np.tile(ob2, 4)[:, None],
            np.tile(v2.T, (4, 1)),
        ], axis=1, dtype=f32)
        im = dict(common)
        im["consts"] = np.ascontiguousarray(cstm)
        in_maps.append(im)
    return in_maps


def _host_combine(outs):
    Z = np.zeros((N + 256, COUT), dtype=np.float64)
    for c in range(NCORES):
        oc = np.asarray(outs[c]).reshape(8, 4, 32, 512)  # [T, g, o, dn]
        for g in range(4):
            seq = oc[:, g].transpose(0, 2, 1).reshape(N, COUT)
            s = 16 * c + 4 * g
            Z[s:s + N] += seq
    return Z[:N].astype(f32)


def kernel(**inputs):
    from concourse import bass_utils

    t = np.asarray(inputs["t"])
    t_min = int(t.min())
    idx = t - t_min
    assert int(t.max()) - t_min + 1 == N, "kernel hardcodes N=4096"

    # scatter observations onto the regular grid (identity when t is arange)
    x_in = np.asarray(inputs["x"], dtype=f32)
    x_new = np.zeros((N, CIN), dtype=f32)
    x_new[idx] = x_in
    ins = dict(inputs)
    ins["x"] = x_new

    if "prog" not in _CACHE:
        _CACHE["prog"] = _build_program()
    nc = _CACHE["prog"]

    in_maps = _host_prep(ins)
    res = bass_utils.run_bass_kernel_spmd(
        nc, in_maps, core_ids=list(range(NCORES)))
    outs = [res.results[c]["out"] for c in range(NCORES)]
    z = _host_combine(outs)
    return z[idx]


if __name__ == "__main__":
    import jax
    cpu = jax.devices("cpu")[0]
    with jax.default_device(cpu):
        sys.path.insert(0, os.path.dirname(os.path.abspath(__file__)))
        import reference as R
        inputs = {k: np.asarray(v) for k, v in R.setup_inputs().items()}
        import jax.numpy as jnp
        z0 = np.asarray(R.reference(**{k: jnp.asarray(v)
                                       for k, v in inputs.items()}))
    z = kernel(**inputs)
    rel = np.linalg.norm(z - z0) / np.linalg.norm(z0)
    print("Relative error:", rel)


# revision 46
# speedup vs baseline: 1.0028x; 1.0028x over previous
"""Trainium2 Bass kernel for nn_CKConv (SIREN kernel-net + causal conv1d).

Decomposition (8 cores, SPMD — identical program, per-core data):
  z[n,o] = sum_{ci, l<=n} W[o,ci,4095-l] * x[n-l,ci],  W[o,ci,m]=weights[m,32o+ci]
Tap l = 128u + 16c + 4g + dl  (c = core, u in [0,32), g,dl in [0,4)).
Each core computes SIREN weights for its 512 taps (SIREN tile cols mm per
block Jb: tap (us,g,dl) at mm=16us+4g+dl, u=8Jb+us), writes layer-3 output
to DRAM, gathers it back as conv lhsT tiles [(32dl+ci), (us,g,o)], and runs
144 accumulating [K=128,M=128,N<=512] matmuls against a 4-shift x image XS.
psum row (g,o) of output tile T holds the contribution to
z[512T+dn+4g+16c, o]; the host sums the shifted partials.

Perf structure: PE warm-up matmuls at t=0 (p-state ramp), sin via the
Activation engine's native Sin after a 4-piece Cody-Waite mod-pi reduction
(parity sign on GPSIMD, off the DVE critical path), layer-3 blocks
interleaved into the conv wave emission so PE never idles on the
W->DRAM->lhsT gather latency, and triangular trim of diagonal conv blocks.

Numerics: layer-1 sin args (up to ~3.4e5 rad) use the reference's exact fp32
rounding sequence; r = a - round(a/pi)*pi via 3 exact 7-bit pieces + 1 fp32
tail (|r| <= pi/2 + 0.021), sin(a) = (-1)^k sin(r).
"""
import os
import sys
import numpy as np

if "/opt/trn_rl_repo" not in sys.path:
    sys.path.insert(0, "/opt/trn_rl_repo")

f32 = np.float32
OMEGA = 32.5
N, CIN, COUT, H = 4096, 32, 32, 32
NCORES = 8
PAD = 512
XSW = PAD + N + 4

MAGIC = 12582912.0  # 1.5 * 2^23
INVPI = float(np.float32(1.0 / np.pi))


def _pieces(val, nexact, bits):
    ps, resid = [], np.float64(val)
    for _ in range(nexact):
        x = np.float32(resid)
        m, e = np.frexp(x)
        q = np.float32(np.ldexp(np.round(np.ldexp(m, bits)), int(e) - bits))
        ps.append(float(q))
        resid = resid - np.float64(q)
    ps.append(float(np.float32(resid)))
    return ps


P1 = _pieces(np.pi, 3, 7)          # |k|<=2^17: 3 exact 7-bit pieces + tail
# |k|<=2^6: single rounded fl32(2pi) piece — residual k*9e-9 rad is
# far inside the sin2 error budget
P2TAU = [float(np.float32(2 * np.pi)), float(np.float32(2*np.pi - np.float64(np.float32(2*np.pi))))]
INV2PI = float(np.float32(1.0 / (2 * np.pi)))

NWARM = 24
NFILL1 = 25
NFILL2 = 45
_CACHE = {}


def _build_program():
    import concourse.bacc as bacc
    import concourse.mybir as mybir
    import concourse.tile as tile

    dt = mybir.dt.float32
    f16 = mybir.dt.float16
    AF = mybir.ActivationFunctionType
    OP = mybir.AluOpType

    nc = bacc.Bacc("TRN2", target_bir_lowering=False, debug=False,
                   num_devices=NCORES)

    d_cst = nc.dram_tensor("consts", [128, 164], dt, kind="ExternalInput")
    d_w3 = nc.dram_tensor("w3rep", [128, 1024], f16, kind="ExternalInput")
    d_b3 = nc.dram_tensor("b3rep", [128, 1024], f16, kind="ExternalInput")
    d_xT = nc.dram_tensor("xT", [32, 4096], f16, kind="ExternalInput")
    d_out = nc.dram_tensor("out", [8, 128, 512], f16, kind="ExternalOutput")

    with tile.TileContext(nc) as tc:
        with (
            tc.tile_pool(name="const", bufs=1) as cp,
            tc.tile_pool(name="sin", bufs=1) as sp,
            tc.tile_pool(name="wsb", bufs=2) as wp,
            tc.tile_pool(name="lhs", bufs=1) as lp,
            tc.tile_pool(name="osb", bufs=4) as op_,
            tc.tile_pool(name="ps3", bufs=4, space="PSUM") as p3p,
            tc.tile_pool(name="psc", bufs=1, space="PSUM") as ppc,
            tc.tile_pool(name="wdram", bufs=1, space="DRAM") as dp,
        ):
            # ---------- t=0: warm-up, act-table preload, memsets, input DMAs
            wz = cp.tile([128, 256], f16, tag="wz")
            nc.gpsimd.memset(wz[:], 0.0)
            zd = sp.tile([128, 1], dt, tag="zd")
            nc.gpsimd.memset(zd[:], 0.0)
            sd = sp.tile([128, 1], dt, tag="sd")
            nc.scalar.activation(sd[:], zd[:], AF.Sin)  # trig table preload
            # warm-up matmuls: keep the PE engine busy (and its p-state
            # ramped) from t~1us until layer-2 inputs arrive
            wps = ppc.tile([128, 512], dt, tag="c0", name="wps")
            for _ in range(NWARM):
                nc.tensor.matmul(wps[:, 0:256], wz[:, 0:128], wz[:, 0:256],
                                 start=True, stop=True)

            cst = cp.tile([128, 164], dt, tag="cst")
            nc.sync.dma_start(cst[:], d_cst[:])
            t1 = cst[:, 0:128]
            w1r = cst[:, 128:129]
            b1r = cst[:, 129:130]
            sc2 = cst[:, 130:131]
            ob2 = cst[:, 131:132]
            v2Tr = cst[:, 132:164]
            w3r = cp.tile([128, 1024], f16, tag="w3r")
            nc.sync.dma_start(w3r[:], d_w3[:])
            b3r = cp.tile([128, 1024], f16, tag="b3r")
            nc.sync.dma_start(b3r[:], d_b3[:])

            # XS: [128=(32dl+ci), XSW];  XS[(dl,ci), PAD+dl+j] = x[j,ci]
            xs = cp.tile([128, XSW], f16, tag="xs")
            nc.gpsimd.memset(xs[:, 0:516], 0.0)
            xs_r = xs[:].rearrange("(dl ci) w -> dl ci w", dl=4)
            for dl in range(4):
                nc.sync.dma_start(
                    xs_r[dl, :, PAD + dl:PAD + dl + 4096], d_xT[:])

            # ---------- sin: DVE Cody-Waite reduce, Act-engine native Sin.
            # mode "parity": reduce mod pi, sign via sin(r*s) = s*sin(r)
            #   with s = (-1)^k, parity from k mod 2 (d^2 is robust to
            #   fmod-vs-floormod negative conventions).
            # mode "clamp": reduce mod 2pi (k slop ~1e-5 rad), clamp to
            #   +-fl32(pi)-ulp so the Sin table range holds.
            PI_IN = float(np.nextafter(np.float32(np.pi), np.float32(0)))

            def dev_sin(out, arg, pieces, inv, nm, parity):
                v = sp.tile([128, 128], dt, tag=f"{nm}v")
                k = sp.tile([128, 128], dt, tag=f"{nm}k")
                r = sp.tile([128, 128], dt, tag=f"{nm}r")
                t_ = sp.tile([128, 128], dt, tag=f"{nm}t")
                nc.vector.tensor_scalar(v[:], arg, inv, MAGIC,
                                        op0=OP.mult, op1=OP.add)
                nc.vector.tensor_scalar_sub(k[:], v[:], MAGIC)
                # piece chain and parity chain are independent given k;
                # interleave them so the in-order DVE pipeline overlaps the
                # dependent-op latencies of both chains.
                piece_ops = []
                sgn = 1.0
                for i, p in enumerate(pieces):
                    dst = r if i % 2 == 0 else t_
                    src = arg if i == 0 else (r if i % 2 == 1 else t_)[:]
                    piece_ops.append((dst, sgn * float(p), src))
                    sgn = -sgn
                cur = (r if (len(pieces) - 1) % 2 == 0 else t_)[:]
                par_ops = []
                if parity:
                    d = sp.tile([128, 128], dt, tag=f"{nm}d")
                    s = sp.tile([128, 128], dt, tag=f"{nm}s")
                    # d = k-2*round(k/2) in {-1,0,1}; s = 1-2*d^2 = (-1)^k
                    par_ops = [
                        lambda: nc.vector.tensor_scalar(
                            d[:], k[:], 0.5, MAGIC, op0=OP.mult, op1=OP.add),
                        lambda: nc.vector.tensor_scalar_sub(d[:], d[:], MAGIC),
                        lambda: nc.vector.scalar_tensor_tensor(
                            d[:], d[:], -2.0, k[:], op0=OP.mult, op1=OP.add),
                        lambda: nc.vector.tensor_mul(d[:], d[:], d[:]),
                        lambda: nc.vector.tensor_scalar(
                            s[:], d[:], -2.0, 1.0, op0=OP.mult, op1=OP.add),
                    ]
                ni = max(len(piece_ops), len(par_ops))
                for i in range(ni):
                    if i < len(par_ops):
                        par_ops[i]()
                    if i < len(piece_ops):
                        dst, p, src = piece_ops[i]
                        nc.vector.scalar_tensor_tensor(
                            dst[:], k[:], p, src,
                            op0=OP.mult, op1=OP.subtract)
                if parity:
                    nc.vector.tensor_mul(cur, cur, s[:])
                else:
                    nc.vector.tensor_scalar(cur, cur, PI_IN, -PI_IN,
                                            op0=OP.min, op1=OP.max)
                nc.scalar.activation(out, cur, AF.Sin)

            # ---------- layer 1:  a1 = fl(fl(fl(t*w1)+b1)*OMEGA)
            a1 = sp.tile([128, 128], dt, tag="a1")
            nc.vector.tensor_scalar(a1[:], t1, w1r, b1r,
                                    op0=OP.mult, op1=OP.add)
            nc.vector.tensor_scalar_mul(a1[:], a1[:], OMEGA)
            h1q = cp.tile([128, 128], dt, tag="h1q")
            dev_sin(h1q[:], a1[:], P1, INVPI, "s1", parity=True)

            # ---------- layer 2 (sca2 = OMEGA*g2/||v2|| from host)
            ps2 = p3p.tile([128, 512], dt, tag="ps3", name="ps2")
            for q in range(4):
                nc.tensor.matmul(ps2[32 * q:32 * q + 32, 0:128],
                                 v2Tr[32 * q:32 * q + 32],
                                 h1q[32 * q:32 * q + 32, :],
                                 start=True, stop=True,
                                 tile_position=(32 * q, 32 * q))
            a2 = sp.tile([128, 128], dt, tag="a2")
            nc.vector.tensor_scalar(a2[:], ps2[:, 0:128], sc2, ob2,
                                    op0=OP.mult, op1=OP.add)
            h2q = cp.tile([128, 128], f16, tag="h2q")
            dev_sin(h2q[:], a2[:], P2TAU, INV2PI, "s2", parity=False)

            # ---------- layer 3 + DRAM-roundtrip gather + conv
            lb = [lp.tile([128, 1024], f16, tag=f"lb{j}", name=f"lb{j}")
                  for j in range(4)]
            wd = [dp.tile([128, 1024], f16, tag=f"wd{j}", name=f"wd{j}")
                  for j in range(4)]

            def l3(Jb):
                pa = p3p.tile([128, 512], dt, tag="ps3")
                pb = p3p.tile([128, 512], dt, tag="ps3")
                for fb, pp in ((0, pa), (1, pb)):
                    nc.tensor.matmul(pp[:],
                                     h2q[32 * Jb:32 * Jb + 32, :],
                                     w3r[32 * Jb:32 * Jb + 32,
                                         512 * fb:512 * fb + 512],
                                     start=True, stop=True,
                                     tile_position=(32 * Jb, 0))
                return pa, pb

            def evac(Jb, pa, pb, engw, engg):
                # wsb = ps3 + b3 (fp16); contiguous wd write per column-half
                # (each starts as soon as its badd lands), then a strided
                # gather back, split by dl-halves across two DMA queues:
                # lb[(32dl+ci), 128us+32g+o] = wd[(16us+4g+dl), 32ci+o]
                # Blocked dma_starts sit on engw/engg's sequencer, so those
                # queues must hold nothing later that is needed earlier.
                wsb = wp.tile([128, 1024], f16, tag=f"wsb{Jb % 2}")
                gsrc = wd[Jb][:].rearrange(
                    "(us g dl) (ci o) -> dl ci us g o", g=4, dl=4, o=32)
                nc.vector.tensor_add(wsb[:, 0:512], pa[:], b3r[:, 0:512])
                if engg is None:
                    engw.dma_start(wd[Jb][:, 0:512], wsb[:, 0:512])
                    nc.vector.tensor_add(wsb[:, 512:1024], pb[:],
                                         b3r[:, 512:1024])
                    engw.dma_start(wd[Jb][:, 512:1024], wsb[:, 512:1024])
                    engw.dma_start(lb[Jb][:], gsrc)
                else:
                    # latency-critical Jb0: disjoint wd writes — a tiny
                    # rows[0:16] (us=0) full-width write plus rows[16:128]
                    # column-halves — so the mini-gather of the first
                    # u-slice waits only on the 32KB write and the conv's
                    # first matmuls start two DMA-stages earlier; the bulk
                    # gather halves exclude us=0 so the u=0 Ldweights only
                    # waits on the mini-gather
                    enga, engb = engg
                    engw.dma_start(wd[Jb][16:128, 0:512], wsb[16:128, 0:512])
                    nc.vector.tensor_add(wsb[:, 512:1024], pb[:],
                                         b3r[:, 512:1024])
                    engw.dma_start(wd[Jb][0:16, :], wsb[0:16, :])
                    engw.dma_start(wd[Jb][16:128, 512:1024],
                                   wsb[16:128, 512:1024])
                    engb.dma_start(lb[Jb][:, 0:128], gsrc[:, :, 0:1])
                    enga.dma_start(lb[Jb][0:64, 128:1024], gsrc[0:2, :, 1:8])
                    engb.dma_start(lb[Jb][64:128, 128:1024],
                                   gsrc[2:4, :, 1:8])

            pscs = {}
            # out-DMA queues: spread so the tail T5/T6/T7 outs don't
            # serialize on one sequencer
            # out DMAs never share a queue with the osb psum-copies (Act):
            # a blocked out-DMA would stall later copies and the B-wave's
            # psum-bank reuse behind them
            # out DMAs never share a queue with the osb psum-copies (Act):
            # a blocked out-DMA would stall later copies and the B-wave's
            # psum-bank reuse behind them.  Pool's SWDGE is slow (~1us
            # descriptor gen) so the tail tiles go on SP.
            oqueue = {0: nc.sync, 1: nc.sync, 2: nc.sync, 3: nc.sync,
                      4: nc.gpsimd, 5: nc.gpsimd, 6: nc.sync, 7: nc.sync}

            def conv_wave(Ts, us_range):
                for u in us_range:
                    Jb, us = u // 8, u % 8
                    lt = lb[Jb][:, 128 * us:128 * us + 128]
                    for T in Ts:
                        nu_T = 4 * (T + 1)
                        if u >= nu_T:
                            continue
                        if T not in pscs:
                            pscs[T] = ppc.tile([128, 512], dt, tag=f"c{T % 4}",
                                               name=f"psc{T}")
                        dn0 = max(0, 128 * (u - 4 * T))
                        off = PAD + 512 * T - 128 * u
                        nc.tensor.matmul(pscs[T][:, dn0:512], lt,
                                         xs[:, off + dn0:off + 512],
                                         start=(u == 0), stop=(u == nu_T - 1),
                                         skip_group_check=True)
                        if u == nu_T - 1:
                            # psum evac on the (otherwise idle) Act engine:
                            # a DVE copy would head-of-line block later badds
                            osb = op_.tile([128, 512], f16, tag=f"o{T % 4}")
                            nc.scalar.activation(osb[:], pscs[T][:],
                                                 AF.Copy)
                            oqueue[T].dma_start(d_out[T], osb[:])

            # filler matmuls keep the PE engine streaming through the psum
            # buffer-rotation waits and the lb0 gather latency, so conv
            # matmuls are dispatched (and costed) against a warm engine
            def filler(n, name):
                fps = ppc.tile([128, 512], dt, tag="c0", name=name)
                for _ in range(n):
                    nc.tensor.matmul(fps[:, 0:256], wz[:, 0:128],
                                     wz[:, 0:256], start=True, stop=True)

            pa0, pb0 = l3(0)
            pa1, pb1 = l3(1)
            evac(0, pa0, pb0, nc.sync, (nc.scalar, nc.sync))
            evac(1, pa1, pb1, nc.sync, None)
            filler(NFILL1, "fps1")
            pa2, pb2 = l3(2)
            pa3, pb3 = l3(3)
            evac(2, pa2, pb2, nc.sync, None)
            evac(3, pa3, pb3, nc.sync, None)
            filler(NFILL2, "fps2")
            conv_wave((0, 1, 2, 3), range(0, 16))
            conv_wave((4, 5, 6, 7), range(0, 32))

    nc.finalize()
    return nc


def _host_prep(inputs):
    """Per-core input maps.  Only consts (t1 packing) differs across cores."""
    import jax
    import jax.numpy as jnp
    cpu = jax.devices("cpu")[0]
    with jax.default_device(cpu):
        t_new = np.asarray(jnp.linspace(-1.0, 2.0 * (N / 1.0) - 1.0, N))
    t_new = t_new.astype(f32)

    x = np.asarray(inputs["x"], dtype=f32)
    v1 = np.asarray(inputs["v1"], dtype=f32)
    g1 = np.asarray(inputs["g1"], dtype=f32)
    b1 = np.asarray(inputs["b1"], dtype=f32)
    v2 = np.asarray(inputs["v2"], dtype=f32)
    g2 = np.asarray(inputs["g2"], dtype=f32)
    b2 = np.asarray(inputs["b2"], dtype=f32)
    w3 = np.asarray(inputs["w3"], dtype=f32)
    b3 = np.asarray(inputs["b3"], dtype=f32)

    w1 = (g1[:, None] * v1 / np.linalg.norm(v1, axis=1, keepdims=True)
          ).astype(f32)[:, 0]
    sca2 = (OMEGA * g2 / np.linalg.norm(v2, axis=1)).astype(f32)
    ob2 = (OMEGA * b2).astype(f32)

    common = {
        "w3rep": np.ascontiguousarray(np.tile(
            np.transpose(w3.reshape(COUT, CIN, H), (2, 1, 0)
                         ).reshape(H, CIN * COUT), (4, 1))).astype(np.float16),
        "b3rep": np.ascontiguousarray(np.tile(
            b3.reshape(COUT, CIN).T.reshape(-1), (128, 1))).astype(np.float16),
        "xT": np.ascontiguousarray(x.T).astype(np.float16),
    }

    mm = np.arange(128)
    g_of = (mm % 16) // 4
    dl_of = mm % 4
    in_maps = []
    for c in range(NCORES):
        t1 = np.empty((128, 128), dtype=f32)
        for Jb in range(4):
            u = 8 * Jb + mm // 16
            m_of = 4095 - 128 * u - 16 * c - 4 * g_of - dl_of
            t1[32 * Jb:32 * Jb + 32, :] = t_new[m_of][None, :]
        cstm = np.concatenate([
            t1,
            np.tile(w1, 4)[:, None], np.tile(b1, 4)[:, None],
            np.tile(sca2, 4)[:, None], np.tile(ob2, 4)[:, None],
            np.tile(v2.T, (4, 1)),
        ], axis=1, dtype=f32)
        im = dict(common)
        im["consts"] = np.ascontiguousarray(cstm)
        in_maps.append(im)
    return in_maps


def _host_combine(outs):
    Z = np.zeros((N + 256, COUT), dtype=np.float64)
    for c in range(NCORES):
        oc = np.asarray(outs[c]).reshape(8, 4, 32, 512)  # [T, g, o, dn]
        for g in range(4):
            seq = oc[:, g].transpose(0, 2, 1).reshape(N, COUT)
            s = 16 * c + 4 * g
            Z[s:s + N] += seq
    return Z[:N].astype(f32)


def kernel(**inputs):
    from concourse import bass_utils

    t = np.asarray(inputs["t"])
    t_min = int(t.min())
    idx = t - t_min
    assert int(t.max()) - t_min + 1 == N, "kernel hardcodes N=4096"

    # scatter observations onto the regular grid (identity when t is arange)
    x_in = np.asarray(inputs["x"], dtype=f32)
    x_new = np.zeros((N, CIN), dtype=f32)
    x_new[idx] = x_in
    ins = dict(inputs)
    ins["x"] = x_new

    if "prog" not in _CACHE:
        _CACHE["prog"] = _build_program()
    nc = _CACHE["prog"]

    in_maps = _host_prep(ins)
    res = bass_utils.run_bass_kernel_spmd(
        nc, in_maps, core_ids=list(range(NCORES)))
    outs = [res.results[c]["out"] for c in range(NCORES)]
    z = _host_combine(outs)
    return z[idx]


if __name__ == "__main__":
    import jax
    cpu = jax.devices("cpu")[0]
    with jax.default_device(cpu):
        sys.path.insert(0, os.path.dirname(os.path.abspath(__file__)))
        import reference as R
        inputs = {k: np.asarray(v) for k, v in R.setup_inputs().items()}
        import jax.numpy as jnp
        z0 = np.asarray(R.reference(**{k: jnp.asarray(v)
                                       for k, v in inputs.items()}))
    z = kernel(**inputs)
    rel = np.linalg.norm(z - z0) / np.linalg.norm(z0)
    print("Relative error:", rel)
